# revision 15
# baseline (speedup 1.0000x reference)
"""Trainium2 Bass kernel for nn_CausalUnlabeled_2044404433206 (moe_routing).

Model per sample:
  e    = emb[f, x_cate[:, f]]                 (16 fields x 8 dims = 128 feats)
  x    = concat(x_cont[64], e[128])           -> 192
  h1   = relu(x @ W1 + b1)                    -> 32
  h2   = relu(h1 @ W2 + b2)                   -> 32
  r    = h2 @ W3 + b3  (NO relu)              -> 32
  hh   = relu(r @ HW1[n] + Hb1[n])            -> [8, 16]
  y    = (hh @ HW2[n] + Hb2[n])[t]

v2 strategy (vs the 177.8 us data-parallel baseline):
  * EXPERT SHARDING: host argsorts samples by routed head t; core c receives
    only head-c samples (65.3k-65.9k for seed 0), padded to C = 34*2048 slots.
    Each core evaluates exactly ONE head -> the all-heads evaluation, the
    one-hot mask, and the gmat select matmul disappear. Output is unpermuted
    host-side (marshalling, like the gather/transposes the baseline did).
  * L3 FUSED INTO H1: r has no relu, so W3@HW1[c] ([32,16]) is precomputed
    host-side (weight-only transform) -> one matmul + one PSUM copy fewer.
  * fp8 eT: embedding values are tiny (s=0.05); ship e4m3(16*e) and fold the
    1/16 into fp16 W1e. Measured rel-err 3.0e-3 (budget 2e-2). x_cont must
    stay fp16 (fp8 there measured 3.7e-2 - it dominates h1 variance).
  * x_cont packed 2 samples/column ([128, C/2] fp16, blockdiag2(W1c)
    stationary) -> halves the L1c stream columns.
  * PAIRED TILES for the head stages: FH1 of tile 2p+1 lands at PSUM
    partitions 64..127 via tile_position (0,64), so one [128,512] relu, one
    [128,8]-stationary FH2 matmul and one y-copy serve both tiles.
  * DMA in ~2.1 MB chunks (>=1 MB for ~340+ GB/s), all issued up front into
    persistent SBUF chunk tiles; warm-up matmuls keep HAM at K=8/8 while the
    first chunk lands.

Per-core per-tile (T=2048, L=512, fold layout [32j+m, k]):
  L1e: 4 col-tiled (0,32j) fp8xfp16 MMs  -> p1 (start)
  L1c: 2 col-tiled (0,64h) packed MMs    -> p1 (accumulate)
  h1 = relu(p1)  [ACT]
  L2:  blockdiag4(W2) [128,128] MM       -> p2 ; h2 = relu(p2)  [DVE]
  FH1: blockdiag4(W3@HW1[c]) [128,64] at (0, 64*(tile%2)) -> ph[pair]
  per pair: hh = relu(ph) [ACT]; FH2 [128,8] MM -> py; y = copy(py) [DVE]; DMA out
"""

import os
import sys

sys.path.insert(0, "/opt/trn_rl_repo")

import numpy as np

B_FULL = 524288
CONT = 64
NF = 16
VOCAB = 1000
EM = 8
RH = 32
RR = 32
PH = 16
NH = 8
N_CORES = 8
T = 2048
LANES = 4
L = T // LANES  # 512
NT = 34  # tiles per core (capacity 69632 >= max head count 65904 + slack)
ESCALE = 16.0  # eT fp8 pre-scale, folded into w1e

_NC_CACHE = {}


def _build(nt, nobias=False):
    from contextlib import ExitStack

    import concourse.mybir as mybir
    import concourse.tile as tile
    from concourse import bacc

    f32 = mybir.dt.float32
    f16 = mybir.dt.float16
    f8 = mybir.dt.float8e4
    AF = mybir.ActivationFunctionType
    OP = mybir.AluOpType

    C = nt * T
    npair = nt // 2
    assert nt % 2 == 0
    # DMA chunk schedule in pairs: small first chunk so compute starts early,
    # then 2-pair (2.1 MB) chunks for bandwidth.
    sched = [1] + [2] * ((npair - 1) // 2)
    if sum(sched) < npair:
        sched.append(npair - sum(sched))
    nchunk = len(sched)
    pair2chunk = []
    for ci, n in enumerate(sched):
        pair2chunk += [ci] * n
    chunk_base = np.concatenate([[0], np.cumsum(sched)])  # in pairs

    nc = bacc.Bacc(
        "TRN2",
        target_bir_lowering=False,
        debug=False,
        enable_asserts=False,
        num_devices=N_CORES,
    )

    d_eT = nc.dram_tensor("eT", [128, C], f8, kind="ExternalInput")
    d_xc2 = nc.dram_tensor("xc2", [128, C // 2], f16, kind="ExternalInput")
    d_w1e = nc.dram_tensor("w1edr", [128, 128], f16, kind="ExternalInput")
    d_w1c2 = nc.dram_tensor("w1c2", [128, 2 * RH], f16, kind="ExternalInput")
    d_w2bd = nc.dram_tensor("w2bd", [128, 128], f16, kind="ExternalInput")
    d_w31 = nc.dram_tensor("w31bd", [128, 64], f16, kind="ExternalInput")
    d_hw2 = nc.dram_tensor("hw2bd8", [128, 8], f16, kind="ExternalInput")
    d_b1 = nc.dram_tensor("b1r", [128, 1], f32, kind="ExternalInput")
    d_b2 = nc.dram_tensor("b2r", [128, 1], f32, kind="ExternalInput")
    d_bh = nc.dram_tensor("bhr", [128, 1], f32, kind="ExternalInput")
    d_bh2 = nc.dram_tensor("bh2r", [8, 1], f32, kind="ExternalInput")
    d_y = nc.dram_tensor("y", [8, npair * L], f32, kind="ExternalOutput")

    with tile.TileContext(nc) as tc, ExitStack() as ctx:
        cpool = ctx.enter_context(tc.tile_pool(name="const", bufs=1))
        apool = ctx.enter_context(tc.tile_pool(name="acts", bufs=3))
        ppool = ctx.enter_context(tc.tile_pool(name="psum", bufs=1, space="PSUM"))

        def cload(dram, shape, dtype, tag):
            tl = cpool.tile(shape, dtype, tag=tag, name=tag)
            nc.sync.dma_start(tl[:], dram.ap())
            return tl

        w1e = cload(d_w1e, [128, 128], f16, "w1e")  # [p, 2, 64]; cols 0-31 = W1e
        w1c2 = cload(d_w1c2, [128, 2 * RH], f16, "w1c2")
        w2bd = cload(d_w2bd, [128, 128], f16, "w2bd")
        w31 = cload(d_w31, [128, 64], f16, "w31")
        hw2 = cload(d_hw2, [128, 8], f16, "hw2")
        b1r = cload(d_b1, [128, 1], f32, "b1r")
        b2r = cload(d_b2, [128, 1], f32, "b2r")
        bhr = cload(d_bh, [128, 1], f32, "bhr")
        bh2r = cload(d_bh2, [8, 1], f32, "bh2r")
        zeros = cpool.tile([128, L], f16, tag="zeros", name="zeros")
        nc.vector.memset(zeros[:], 0.0)

        # ---- input chunks: issue every DMA up front, interleaved e/x ----
        ech, xch = [], []
        for k in range(nchunk):
            lo = int(chunk_base[k]) * 2 * T
            hi = int(chunk_base[k + 1]) * 2 * T
            w = hi - lo
            et = cpool.tile([128, w], f8, tag=f"ec{k}", name=f"ec{k}")
            nc.sync.dma_start(et[:], d_eT.ap()[:, lo:hi])
            xt = cpool.tile([128, w // 2], f16, tag=f"xc{k}", name=f"xc{k}")
            nc.sync.dma_start(xt[:], d_xc2.ap()[:, lo // 2 : hi // 2])
            ech.append(et)
            xch.append(xt)

        ybuf = cpool.tile([8, npair * L], f32, tag="ybuf", name="ybuf")

        # ---- HAM warm-up: ~16 N=512 matmuls on zeros while chunk 0 lands ----
        for w in range(16):
            pw = ppool.tile([8, L], f32, tag="py", bufs=2, name=f"warm{w}")
            nc.tensor.matmul(pw[:], hw2[:], zeros[:], start=True, stop=True)

        # fp8 stationary [128, 32] (normal mode; DoubleRow rejects the
        # (0,64) dst placement the fold layout needs)
        w1e3 = w1e[:, :RH]

        for p in range(npair):
            k = pair2chunk[p]
            ec, xc = ech[k], xch[k]
            poff = (p - int(chunk_base[k])) * 2 * T  # eT col offset of tile 2p

            hpair = []
            for s in (0, 1):  # the two tiles of the pair
                eo = poff + s * T
                xo = (poff + s * T) // 2
                # ---- L1: DoubleRow fp8 e-part (2 lanes per MM) + packed c ----
                p1 = ppool.tile([128, L], f32, tag="p1", bufs=2, name=f"p1_{p}_{s}")
                for j in range(LANES):
                    nc.tensor.matmul(
                        p1[32 * j : 32 * j + 32, :],
                        w1e3,
                        ec[:, eo + j * L : eo + (j + 1) * L],
                        start=True, stop=False, tile_position=(0, 32 * j),
                        skip_group_check=True,
                    )
                for h in (0, 1):
                    nc.tensor.matmul(
                        p1[64 * h : 64 * h + 64, :],
                        w1c2[:],
                        xc[:, xo + h * L : xo + (h + 1) * L],
                        start=False, stop=True, tile_position=(0, 64 * h),
                        skip_group_check=True,
                    )
                h1 = apool.tile([128, L], f16, tag="h1", name=f"h1_{p}_{s}")
                if nobias:
                    nc.scalar.activation(h1[:], p1[:], AF.Relu, scale=1.0 / ESCALE)
                else:
                    nc.scalar.activation(
                        h1[:], p1[:], AF.Relu, bias=b1r[:], scale=1.0 / ESCALE
                    )

                # ---- L2 ----
                p2 = ppool.tile([128, L], f32, tag="p2", bufs=2, name=f"p2_{p}_{s}")
                nc.tensor.matmul(p2[:], w2bd[:], h1[:], start=True, stop=True)
                h2 = apool.tile([128, L], f16, tag="h2", name=f"h2_{p}_{s}")
                if nobias:
                    nc.vector.tensor_scalar_max(h2[:], p2[:], 0.0)
                else:
                    nc.vector.scalar_tensor_tensor(
                        h2[:], p2[:], b2r[:], zeros[:], OP.add, OP.max
                    )
                hpair.append(h2)

            # ---- FH1: both tiles into one PSUM tile (cols 0-1 / 2-3) ----
            ph = ppool.tile([128, L], f32, tag="ph", bufs=2, name=f"ph_{p}")
            for s in (0, 1):
                nc.tensor.matmul(
                    ph[64 * s : 64 * s + 64, :],
                    w31[:],
                    hpair[s][:],
                    start=True, stop=True, tile_position=(0, 64 * s),
                    skip_group_check=True,
                )
            hh = apool.tile([128, L], f16, tag="hh", name=f"hh_{p}")
            if nobias:
                nc.scalar.activation(hh[:], ph[:], AF.Relu)
            else:
                nc.scalar.activation(hh[:], ph[:], AF.Relu, bias=bhr[:])

            # ---- FH2: one [128,8] matmul for the pair ----
            py = ppool.tile([8, L], f32, tag="py", bufs=2, name=f"py_{p}")
            nc.tensor.matmul(py[:], hw2[:], hh[:], start=True, stop=True)
            ysl = ybuf[:, p * L : (p + 1) * L]
            if nobias:
                nc.vector.tensor_copy(ysl, py[:])
            else:
                nc.vector.scalar_tensor_tensor(
                    ysl, py[:], bh2r[:], zeros[:8, :], OP.add, OP.add
                )
            # flush output in two halves so the tail DMA overlaps compute
            if p == npair // 2:
                nc.sync.dma_start(
                    d_y.ap()[:, : (p + 1) * L], ybuf[:, : (p + 1) * L]
                )
            elif p == npair - 1:
                nc.sync.dma_start(
                    d_y.ap()[:, (npair // 2 + 1) * L :],
                    ybuf[:, (npair // 2 + 1) * L :],
                )

    nc.compile()
    return nc


def _host_prep(x_cont, x_cate, t, emb, W1, b1, W2, b2, W3, b3, HW1, Hb1, HW2, Hb2, nt):
    """Per-core input maps: permutation by routed head + weight reshapes."""
    import ml_dtypes

    f16 = np.float16
    f32 = np.float32
    e4 = ml_dtypes.float8_e4m3
    C = nt * T
    B = x_cont.shape[0]

    tt = t.reshape(-1).astype(np.int64)
    order = np.argsort(tt, kind="stable")
    counts = np.bincount(tt, minlength=NH)
    assert counts.max() <= C, (counts.max(), C)
    starts = np.concatenate([[0], np.cumsum(counts)])

    # ---- shared weights ----
    # DoubleRow stationary [p, 2, 64]: set 0 -> out cols 0-31 (even lane),
    # set 1 -> out cols 32-63 (odd lane). PSUM carries ESCALE*h1pre; the
    # c-part weights are scaled up to match, descaled in the h1 activation.
    w1e8 = W1[CONT:]  # [128, 32], fp16 stationary (fp8 stationary hurt MM pairing)
    w1edr = np.zeros((128, 2, 64), f32)
    w1edr[:, 0, :RH] = w1e8
    w1edr[:, 1, RH:] = w1e8
    w1edr = w1edr.reshape(128, 128).astype(f16)
    w1c2 = np.zeros((128, 2 * RH), f32)
    w1c2[:CONT, :RH] = W1[:CONT] * ESCALE
    w1c2[CONT:, RH:] = W1[:CONT] * ESCALE
    w1c2 = w1c2.astype(f16)

    def blockdiag4(w):
        k, m = w.shape
        out = np.zeros((4 * k, 4 * m), f32)
        for j in range(4):
            out[k * j : k * (j + 1), m * j : m * (j + 1)] = w
        return out

    w2bd = blockdiag4(W2).astype(f16)
    b1r = np.tile(b1, 4).astype(f32)[:, None]
    b2r = np.tile(b2, 4).astype(f32)[:, None]

    # ---- embedding rows fp8, scaled: eT[f*8+d, slot] ----
    flat8 = (emb.reshape(NF * VOCAB, EM) * ESCALE).astype(e4)
    idx_flat = x_cate.astype(np.int64) + (np.arange(NF) * VOCAB)[None, :]

    xc16 = x_cont.astype(f16)

    in_maps = []
    perms = []
    for c in range(NH):
        perm = np.zeros(C, np.int64)
        cnt = counts[c]
        perm[:cnt] = order[starts[c] : starts[c + 1]]
        perms.append(perm)

        e_rows = flat8[idx_flat[perm]].reshape(C, NF * EM)  # [C,128] e4m3
        eT = np.ascontiguousarray(e_rows.T)

        xcp = xc16[perm]  # [C, 64]
        xv = xcp.reshape(nt, 2, 2, L, CONT)  # [tile, h, sub, k, feat]
        xc2 = np.empty((128, C // 2), f16)
        x2v = xc2.reshape(2, CONT, nt, 2, L)  # [sub, feat, tile, h, k]
        x2v[0] = xv[:, :, 0].transpose(3, 0, 1, 2)
        x2v[1] = xv[:, :, 1].transpose(3, 0, 1, 2)

        w31 = blockdiag4(W3 @ HW1[c]).astype(f16)  # [128, 64]
        hw2bd8 = np.zeros((128, 8), f32)
        bhr = np.zeros((128, 1), f32)
        bh1c = Hb1[c] + b3 @ HW1[c]  # fused FH1 bias [16]
        for s in (0, 1):
            for j in range(LANES):
                r0 = 64 * s + PH * j
                hw2bd8[r0 : r0 + PH, 4 * s + j] = HW2[c, :, 0]
            bhr[64 * s : 64 * s + 64, 0] = np.tile(bh1c, 4)
        bh2r = np.full((8, 1), Hb2[c, 0], f32)

        in_maps.append(
            dict(
                eT=eT, xc2=xc2, w1edr=w1edr, w1c2=w1c2, w2bd=w2bd,
                w31bd=w31, hw2bd8=hw2bd8.astype(f16),
                b1r=b1r, b2r=b2r, bhr=bhr, bh2r=bh2r,
            )
        )
    return in_maps, perms, counts


def kernel(**inputs):
    from concourse.bass_utils import run_bass_kernel_spmd

    x_cont = np.asarray(inputs["x_cont"], dtype=np.float32)
    x_cate = np.asarray(inputs["x_cate"])
    t = np.asarray(inputs["t"])
    emb = np.asarray(inputs["emb"], dtype=np.float32)
    args = [np.asarray(inputs[k], dtype=np.float32) for k in
            ("W1", "b1", "W2", "b2", "W3", "b3", "HW1", "Hb1", "HW2", "Hb2")]

    B = x_cont.shape[0]
    tt = np.asarray(t).reshape(-1).astype(np.int64)
    nt = NT
    maxc = np.bincount(tt, minlength=NH).max()
    while nt * T < maxc:  # adversarial t safety: grow capacity
        nt += 2

    in_maps, perms, counts = _host_prep(x_cont, x_cate, t, emb, *args, nt=nt)

    b1, b2, b3, Hb1, Hb2 = args[1], args[3], args[5], args[7], args[9]
    nobias = all(not np.any(x) for x in (b1, b2, b3, Hb1, Hb2))
    key = (nt, nobias)
    if key not in _NC_CACHE:
        _NC_CACHE[key] = _build(nt, nobias=nobias)
    nc = _NC_CACHE[key]

    trace = os.environ.get("KERNEL_TRACE", "0") == "1"
    res = run_bass_kernel_spmd(nc, in_maps, core_ids=list(range(N_CORES)), trace=trace)
    global LAST
    LAST = res
    y = np.empty(B, np.float32)
    npair = nt // 2
    for c in range(NH):
        # d_y is [8, npair*512]: row 4*s+j, col p*512+k  <->  slot
        # (2p+s)*2048 + j*512 + k; transpose to slot order.
        yc = np.asarray(res.results[c]["y"]).reshape(8, npair, L)
        yc = yc.transpose(1, 0, 2).reshape(-1)
        cnt = counts[c]
        y[perms[c][:cnt]] = yc[:cnt]
    return y


LAST = None


# revision 16
# speedup vs baseline: 1.1228x; 1.1228x over previous
"""Trainium2 Bass kernel for nn_CausalUnlabeled_2044404433206 (moe_routing).

Model per sample:
  e    = emb[f, x_cate[:, f]]                 (16 fields x 8 dims = 128 feats)
  x    = concat(x_cont[64], e[128])           -> 192
  h1   = relu(x @ W1 + b1)                    -> 32
  h2   = relu(h1 @ W2 + b2)                   -> 32
  r    = h2 @ W3 + b3  (NO relu)              -> 32
  hh   = relu(r @ HW1[n] + Hb1[n])            -> [8, 16]
  y    = (hh @ HW2[n] + Hb2[n])[t]

v2 strategy (vs the 177.8 us data-parallel baseline):
  * EXPERT SHARDING: host argsorts samples by routed head t; core c receives
    only head-c samples (65.3k-65.9k for seed 0), padded to C = 34*2048 slots.
    Each core evaluates exactly ONE head -> the all-heads evaluation, the
    one-hot mask, and the gmat select matmul disappear. Output is unpermuted
    host-side (marshalling, like the gather/transposes the baseline did).
  * L3 FUSED INTO H1: r has no relu, so W3@HW1[c] ([32,16]) is precomputed
    host-side (weight-only transform) -> one matmul + one PSUM copy fewer.
  * fp8 eT: embedding values are tiny (s=0.05); ship e4m3(16*e) and fold the
    1/16 into fp16 W1e. Measured rel-err 3.0e-3 (budget 2e-2). x_cont must
    stay fp16 (fp8 there measured 3.7e-2 - it dominates h1 variance).
  * x_cont packed 2 samples/column ([128, C/2] fp16, blockdiag2(W1c)
    stationary) -> halves the L1c stream columns.
  * PAIRED TILES for the head stages: FH1 of tile 2p+1 lands at PSUM
    partitions 64..127 via tile_position (0,64), so one [128,512] relu, one
    [128,8]-stationary FH2 matmul and one y-copy serve both tiles.
  * DMA in ~2.1 MB chunks (>=1 MB for ~340+ GB/s), all issued up front into
    persistent SBUF chunk tiles; warm-up matmuls keep HAM at K=8/8 while the
    first chunk lands.

Per-core per-tile (T=2048, L=512, fold layout [32j+m, k]):
  L1e: 4 col-tiled (0,32j) fp8xfp16 MMs  -> p1 (start)
  L1c: 2 col-tiled (0,64h) packed MMs    -> p1 (accumulate)
  h1 = relu(p1)  [ACT]
  L2:  blockdiag4(W2) [128,128] MM       -> p2 ; h2 = relu(p2)  [DVE]
  FH1: blockdiag4(W3@HW1[c]) [128,64] at (0, 64*(tile%2)) -> ph[pair]
  per pair: hh = relu(ph) [ACT]; FH2 [128,8] MM -> py; y = copy(py) [DVE]; DMA out
"""

import os
import sys

sys.path.insert(0, "/opt/trn_rl_repo")

import numpy as np

B_FULL = 524288
CONT = 64
NF = 16
VOCAB = 1000
EM = 8
RH = 32
RR = 32
PH = 16
NH = 8
N_CORES = 8
T = 2048
LANES = 4
L = T // LANES  # 512
NT = 34  # tiles per core (capacity 69632 >= max head count 65904 + slack)
ESCALE = 16.0  # eT fp8 pre-scale, folded into w1e

_NC_CACHE = {}


def _build(nt, nobias=False):
    from contextlib import ExitStack

    import concourse.mybir as mybir
    import concourse.tile as tile
    from concourse import bacc

    f32 = mybir.dt.float32
    f16 = mybir.dt.float16
    f8 = mybir.dt.float8e4
    AF = mybir.ActivationFunctionType
    OP = mybir.AluOpType

    C = nt * T
    npair = nt // 2
    assert nt % 2 == 0
    # DMA chunk schedule in pairs: 1-pair first chunk so compute starts
    # early, then 2-pair (2.1 MB) chunks for bandwidth.
    sched = [1] + [2] * ((npair - 1) // 2)
    if sum(sched) < npair:
        sched.append(npair - sum(sched))
    nchunk = len(sched)
    pair2chunk = []
    for ci, n in enumerate(sched):
        pair2chunk += [ci] * n
    chunk_base = np.concatenate([[0], np.cumsum(sched)]).astype(int)

    nc = bacc.Bacc(
        "TRN2",
        target_bir_lowering=False,
        debug=False,
        enable_asserts=False,
        num_devices=N_CORES,
    )

    d_eT = nc.dram_tensor("eT", [128, C], f8, kind="ExternalInput")
    d_xc2 = nc.dram_tensor("xc2", [128, C // 2], f16, kind="ExternalInput")
    d_w1e = nc.dram_tensor("w1e", [128, RH], f16, kind="ExternalInput")
    d_w1c2 = nc.dram_tensor("w1c2", [128, 2 * RH], f16, kind="ExternalInput")
    d_w2bd = nc.dram_tensor("w2bd", [128, 128], f16, kind="ExternalInput")
    d_w31 = nc.dram_tensor("w31bd", [128, 64], f16, kind="ExternalInput")
    d_hw2 = nc.dram_tensor("hw2bd8", [128, 8], f16, kind="ExternalInput")
    d_b1 = nc.dram_tensor("b1r", [128, 1], f32, kind="ExternalInput")
    d_b2 = nc.dram_tensor("b2r", [128, 1], f32, kind="ExternalInput")
    d_bh = nc.dram_tensor("bhr", [128, 1], f32, kind="ExternalInput")
    d_bh2 = nc.dram_tensor("bh2r", [8, 1], f32, kind="ExternalInput")
    d_y = nc.dram_tensor("y", [8 * npair, L], f32, kind="ExternalOutput")

    with tile.TileContext(nc) as tc, ExitStack() as ctx:
        cpool = ctx.enter_context(tc.tile_pool(name="const", bufs=1))
        apool = ctx.enter_context(tc.tile_pool(name="acts", bufs=3))
        ppool = ctx.enter_context(tc.tile_pool(name="psum", bufs=1, space="PSUM"))

        def cload(dram, shape, dtype, tag):
            tl = cpool.tile(shape, dtype, tag=tag, name=tag)
            nc.sync.dma_start(tl[:], dram.ap())
            return tl

        w1e = cload(d_w1e, [128, RH], f16, "w1e")
        w1c2 = cload(d_w1c2, [128, 2 * RH], f16, "w1c2")
        w2bd = cload(d_w2bd, [128, 128], f16, "w2bd")
        w31 = cload(d_w31, [128, 64], f16, "w31")
        hw2 = cload(d_hw2, [128, 8], f16, "hw2")
        b1r = cload(d_b1, [128, 1], f32, "b1r")
        b2r = cload(d_b2, [128, 1], f32, "b2r")
        bhr = cload(d_bh, [128, 1], f32, "bhr")
        bh2r = cload(d_bh2, [8, 1], f32, "bh2r")
        zeros = cpool.tile([128, L], f16, tag="zeros", name="zeros")
        nc.vector.memset(zeros[:], 0.0)

        # ---- input chunks: issue every DMA up front, interleaved e/x ----
        ech, xch = [], []
        for k in range(nchunk):
            lo = int(chunk_base[k]) * 2 * T
            hi = int(chunk_base[k + 1]) * 2 * T
            w = hi - lo
            et = cpool.tile([128, w], f8, tag=f"ec{k}", name=f"ec{k}")
            nc.sync.dma_start(et[:], d_eT.ap()[:, lo:hi])
            xt = cpool.tile([128, w // 2], f16, tag=f"xc{k}", name=f"xc{k}")
            nc.sync.dma_start(xt[:], d_xc2.ap()[:, lo // 2 : hi // 2])
            ech.append(et)
            xch.append(xt)

        # ---- HAM warm-up: ~16 N=512 matmuls on zeros while chunk 0 lands ----
        for w in range(16):
            pw = ppool.tile([8, L], f32, tag="py", bufs=2, name=f"warm{w}")
            nc.tensor.matmul(pw[:], hw2[:], zeros[:], start=True, stop=True)

        for p in range(npair):
            k = pair2chunk[p]
            ec, xc = ech[k], xch[k]
            poff = (p - int(chunk_base[k])) * 2 * T  # eT col offset of tile 2p

            hpair = []
            for s in (0, 1):  # the two tiles of the pair
                eo = poff + s * T
                xo = (poff + s * T) // 2
                # ---- L1 ----
                p1 = ppool.tile([128, L], f32, tag="p1", bufs=2, name=f"p1_{p}_{s}")
                for j in range(LANES):
                    nc.tensor.matmul(
                        p1[32 * j : 32 * j + 32, :],
                        w1e[:],
                        ec[:, eo + j * L : eo + (j + 1) * L],
                        start=True, stop=False, tile_position=(0, 32 * j),
                        skip_group_check=True,
                    )
                for h in (0, 1):
                    nc.tensor.matmul(
                        p1[64 * h : 64 * h + 64, :],
                        w1c2[:],
                        xc[:, xo + h * L : xo + (h + 1) * L],
                        start=False, stop=True, tile_position=(0, 64 * h),
                        skip_group_check=True,
                    )
                h1 = apool.tile([128, L], f16, tag="h1", name=f"h1_{p}_{s}")
                if nobias:
                    nc.scalar.activation(h1[:], p1[:], AF.Relu)
                else:
                    nc.scalar.activation(h1[:], p1[:], AF.Relu, bias=b1r[:])

                # ---- L2 ----
                p2 = ppool.tile([128, L], f32, tag="p2", bufs=2, name=f"p2_{p}_{s}")
                nc.tensor.matmul(p2[:], w2bd[:], h1[:], start=True, stop=True)
                h2 = apool.tile([128, L], f16, tag="h2", name=f"h2_{p}_{s}")
                if nobias:
                    nc.vector.tensor_scalar_max(h2[:], p2[:], 0.0)
                else:
                    nc.vector.scalar_tensor_tensor(
                        h2[:], p2[:], b2r[:], zeros[:], OP.add, OP.max
                    )
                hpair.append(h2)

            # ---- FH1: both tiles into one PSUM tile (cols 0-1 / 2-3) ----
            ph = ppool.tile([128, L], f32, tag="ph", bufs=2, name=f"ph_{p}")
            for s in (0, 1):
                nc.tensor.matmul(
                    ph[64 * s : 64 * s + 64, :],
                    w31[:],
                    hpair[s][:],
                    start=True, stop=True, tile_position=(0, 64 * s),
                    skip_group_check=True,
                )
            hh = apool.tile([128, L], f16, tag="hh", name=f"hh_{p}")
            if nobias:
                nc.scalar.activation(hh[:], ph[:], AF.Relu)
            else:
                nc.scalar.activation(hh[:], ph[:], AF.Relu, bias=bhr[:])

            # ---- FH2: one [128,8] matmul for the pair ----
            py = ppool.tile([8, L], f32, tag="py", bufs=2, name=f"py_{p}")
            nc.tensor.matmul(py[:], hw2[:], hh[:], start=True, stop=True)
            ysb = apool.tile([8, L], f32, tag="ysb", name=f"ysb_{p}")
            if nobias:
                nc.vector.tensor_copy(ysb[:], py[:])
            else:
                nc.vector.scalar_tensor_tensor(
                    ysb[:], py[:], bh2r[:], zeros[:8, :], OP.add, OP.add
                )
            nc.sync.dma_start(d_y.ap()[8 * p : 8 * p + 8, :], ysb[:])

    nc.compile()
    return nc


def _host_prep(x_cont, x_cate, t, emb, W1, b1, W2, b2, W3, b3, HW1, Hb1, HW2, Hb2, nt):
    """Per-core input maps: permutation by routed head + weight reshapes."""
    import ml_dtypes

    f16 = np.float16
    f32 = np.float32
    e4 = ml_dtypes.float8_e4m3
    C = nt * T
    B = x_cont.shape[0]

    tt = t.reshape(-1).astype(np.int64)
    order = np.argsort(tt, kind="stable")
    counts = np.bincount(tt, minlength=NH)
    assert counts.max() <= C, (counts.max(), C)
    starts = np.concatenate([[0], np.cumsum(counts)])

    # ---- shared weights ----
    w1e = (W1[CONT:] / ESCALE).astype(f16)  # [128, 32], rows (f*8+d)
    w1c2 = np.zeros((128, 2 * RH), f32)
    w1c2[:CONT, :RH] = W1[:CONT]
    w1c2[CONT:, RH:] = W1[:CONT]
    w1c2 = w1c2.astype(f16)

    def blockdiag4(w):
        k, m = w.shape
        out = np.zeros((4 * k, 4 * m), f32)
        for j in range(4):
            out[k * j : k * (j + 1), m * j : m * (j + 1)] = w
        return out

    w2bd = blockdiag4(W2).astype(f16)
    b1r = np.tile(b1, 4).astype(f32)[:, None]
    b2r = np.tile(b2, 4).astype(f32)[:, None]

    # ---- embedding rows fp8, scaled: eT[f*8+d, slot] ----
    flat8 = (emb.reshape(NF * VOCAB, EM) * ESCALE).astype(e4)
    idx_flat = x_cate.astype(np.int64) + (np.arange(NF) * VOCAB)[None, :]

    xc16 = x_cont.astype(f16)

    in_maps = []
    perms = []
    for c in range(NH):
        perm = np.zeros(C, np.int64)
        cnt = counts[c]
        perm[:cnt] = order[starts[c] : starts[c + 1]]
        perms.append(perm)

        e_rows = flat8[idx_flat[perm]].reshape(C, NF * EM)  # [C,128] e4m3
        eT = np.ascontiguousarray(e_rows.T)

        xcp = xc16[perm]  # [C, 64]
        xv = xcp.reshape(nt, 2, 2, L, CONT)  # [tile, h, sub, k, feat]
        xc2 = np.empty((128, C // 2), f16)
        x2v = xc2.reshape(2, CONT, nt, 2, L)  # [sub, feat, tile, h, k]
        x2v[0] = xv[:, :, 0].transpose(3, 0, 1, 2)
        x2v[1] = xv[:, :, 1].transpose(3, 0, 1, 2)

        w31 = blockdiag4(W3 @ HW1[c]).astype(f16)  # [128, 64]
        hw2bd8 = np.zeros((128, 8), f32)
        bhr = np.zeros((128, 1), f32)
        bh1c = Hb1[c] + b3 @ HW1[c]  # fused FH1 bias [16]
        for s in (0, 1):
            for j in range(LANES):
                r0 = 64 * s + PH * j
                hw2bd8[r0 : r0 + PH, 4 * s + j] = HW2[c, :, 0]
            bhr[64 * s : 64 * s + 64, 0] = np.tile(bh1c, 4)
        bh2r = np.full((8, 1), Hb2[c, 0], f32)

        in_maps.append(
            dict(
                eT=eT, xc2=xc2, w1e=w1e, w1c2=w1c2, w2bd=w2bd,
                w31bd=w31, hw2bd8=hw2bd8.astype(f16),
                b1r=b1r, b2r=b2r, bhr=bhr, bh2r=bh2r,
            )
        )
    return in_maps, perms, counts


def kernel(**inputs):
    from concourse.bass_utils import run_bass_kernel_spmd

    x_cont = np.asarray(inputs["x_cont"], dtype=np.float32)
    x_cate = np.asarray(inputs["x_cate"])
    t = np.asarray(inputs["t"])
    emb = np.asarray(inputs["emb"], dtype=np.float32)
    args = [np.asarray(inputs[k], dtype=np.float32) for k in
            ("W1", "b1", "W2", "b2", "W3", "b3", "HW1", "Hb1", "HW2", "Hb2")]

    B = x_cont.shape[0]
    tt = np.asarray(t).reshape(-1).astype(np.int64)
    nt = NT
    maxc = np.bincount(tt, minlength=NH).max()
    while nt * T < maxc:  # adversarial t safety: grow capacity
        nt += 2

    in_maps, perms, counts = _host_prep(x_cont, x_cate, t, emb, *args, nt=nt)

    b1, b2, b3, Hb1, Hb2 = args[1], args[3], args[5], args[7], args[9]
    nobias = all(not np.any(x) for x in (b1, b2, b3, Hb1, Hb2))
    key = (nt, nobias)
    if key not in _NC_CACHE:
        _NC_CACHE[key] = _build(nt, nobias=nobias)
    nc = _NC_CACHE[key]

    trace = os.environ.get("KERNEL_TRACE", "0") == "1"
    res = run_bass_kernel_spmd(nc, in_maps, core_ids=list(range(N_CORES)), trace=trace)
    global LAST
    LAST = res
    y = np.empty(B, np.float32)
    for c in range(NH):
        yc = res.results[c]["y"].reshape(-1)
        cnt = counts[c]
        y[perms[c][:cnt]] = yc[:cnt]
    return y


LAST = None
